# revision 1
# baseline (speedup 1.0000x reference)
"""CrossAttention Trainium2 kernel (Bass/Tile), 8-core SPMD.

Problem: q = query@Wq+bq; k = key@Wk+bk; v = value@Wv+bv;
         out = softmax(q k^T) v           (no 1/sqrt(d) scaling)
Shapes:  query [4, 2048, 1024], key/value [4, 2048, 768],
         W* [(1024|768), 1024], b* [1024], out [4, 2048, 1024] f32.

Sharding: data-parallel over (batch, query-half) -> 8 shards of 1024 query
rows. Each core redundantly projects its batch's full K/V (no collectives).

Layout: the host pre-transposes query/key/value to feature-major so the PE
contraction dim lands on partitions with plain DMAs (no on-chip input
transposes). Only the softmax-probability transpose runs on the PE.

Precision: projections + scores run the PE in float32r (rounded fp32,
1 cyc/row at N>=512; measured logit abs err ~5e-3 on sigma=32 logits);
softmax probs and V are bf16 for the final GEMM (linear error, ~2^-9).

SBUF/overlap strategy: only kT (64KB/part) and v (32KB/part) stay
SBUF-resident; qT spills to internal DRAM during stage A and streams back
per m-tile in stage D. Every stage's working set leaves >40KB/partition
free so the next stage's DMA prefetch never waits on a dying tile (WAR).
The attention m-loop is software-pipelined (AV of m-tile i runs on the PE
while softmax of m-tile i+1 occupies ACT/DVE).
"""

import os
import sys
from contextlib import ExitStack

for _p in ("/opt/trn_rl_repo", "/root/.axon_site/_ro/trn_rl_repo"):
    if os.path.isdir(_p) and _p not in sys.path:
        sys.path.append(_p)

import numpy as np

import concourse.bass as bass
import concourse.mybir as mybir
import concourse.tile as tile
from concourse import bacc
from concourse.bass import ts
from concourse.bass_utils import run_bass_kernel_spmd
from concourse.masks import make_identity

P = 128
B, LQ, LK = 4, 2048, 2048
D1, D2, H = 1024, 768, 1024
N_CORES = 8
M = (B * LQ) // N_CORES  # 1024 query rows per core

D1T, D2T, HT, MT, JT, JC = D1 // P, D2 // P, H // P, M // P, LK // P, LK // 512

F32 = mybir.dt.float32
F32R = mybir.dt.float32r
BF16 = mybir.dt.bfloat16
AX = mybir.AxisListType.X
AF = mybir.ActivationFunctionType
ALU = mybir.AluOpType

_CACHE = {}
LAST_RESULTS = None  # BassKernelResults of the most recent run (for test harness)


def _build_bass():
    nc = bacc.Bacc("TRN2", target_bir_lowering=False, debug=False,
                   num_devices=N_CORES)

    # All big operands arrive feature-major (pre-transposed on the host).
    xqt = nc.dram_tensor("xqt", [D1, M], F32R, kind="ExternalInput")
    kyt = nc.dram_tensor("kyt", [D2, LK], F32R, kind="ExternalInput")
    vvt = nc.dram_tensor("vvt", [D2, LK], BF16, kind="ExternalInput")
    wq = nc.dram_tensor("wq", [D1, H], F32R, kind="ExternalInput")
    wk = nc.dram_tensor("wk", [D2, H], F32R, kind="ExternalInput")
    wv = nc.dram_tensor("wv", [D2, H], BF16, kind="ExternalInput")
    bqd = nc.dram_tensor("bq", [H], F32, kind="ExternalInput")
    bkd = nc.dram_tensor("bk", [H], F32, kind="ExternalInput")
    bvd = nc.dram_tensor("bv", [H], F32, kind="ExternalInput")
    out = nc.dram_tensor("out", [M, H], F32, kind="ExternalOutput")

    wq_t = wq.rearrange("(t p) h -> p t h", p=P)
    wk_t = wk.rearrange("(t p) h -> p t h", p=P)
    wv_t = wv.rearrange("(t p) h -> p t h", p=P)
    xqt_t = xqt.rearrange("(t p) m -> p t m", p=P)
    kyt_t = kyt.rearrange("(t p) j -> p t j", p=P)
    vvt_t = vvt.rearrange("(t p) j -> p t j", p=P)

    with tile.TileContext(nc) as tc, ExitStack() as top:
        const = top.enter_context(tc.tile_pool(name="const", bufs=1))
        bias2 = const.tile([P, 2, HT], F32)
        nc.sync.dma_start(bias2[:, 0, :], bqd.rearrange("(t p) -> p t", p=P))
        nc.sync.dma_start(bias2[:, 1, :], bkd.rearrange("(t p) -> p t", p=P))
        bqt = bias2[:, 0, :]
        bkt = bias2[:, 1, :]

        # Shared PSUM accumulation pool (scores / projections / AV).
        pps = top.enter_context(tc.tile_pool(name="pps", bufs=5, space="PSUM"))

        # Residents: qT [H, M] + kT [H, LK] f32r.
        respool = top.enter_context(tc.tile_pool(name="res", bufs=1))
        qT = respool.tile([P, HT, M], F32R)
        kT = respool.tile([P, HT, LK], F32R)

        # Stage-B pools live on the RIGHT SBUF stack and are opened before
        # stage A so their prefetch DMAs don't inherit a WAR dependency on
        # stage A's buffers (the left stack rewinds into A's range).
        esB = top.enter_context(ExitStack())
        sb1 = esB.enter_context(tc.tile_pool(name="sb1", bufs=1, side="right"))
        sb3 = esB.enter_context(tc.tile_pool(name="sb3", bufs=2, side="right"))
        wk_h0 = sb1.tile([P, D2T // 2, H], F32R)

        # ---- Stage A: qT[h, m] = Wq^T @ X^T + bq ----
        # One need-ordered queue (sync): wq first half, X^T, wq second
        # half, then B's weights — so the PE is fed from ~7us on.
        with tc.tile_pool(name="sa1", bufs=1, side="left") as sa1:
            wqs = sa1.tile([P, D1T, H], F32R)
            xTs = sa1.tile([P, D1T, M], F32R)
            nc.sync.dma_start(wqs[:, :, 0:512], wq_t[:, :, 0:512])
            for dt in range(D1T):
                nc.sync.dma_start(xTs[:, dt, :], xqt_t[:, dt, :])
            nc.sync.dma_start(wqs[:, :, 512:1024], wq_t[:, :, 512:1024])
            nc.sync.dma_start(wk_h0[:], wk_t[:, 0:D2T // 2, :])
            for ht in range(HT):
                for mc in range(M // 512):
                    psq = pps.tile([P, 512], F32, tag="acc")
                    for dt in range(D1T):
                        nc.tensor.matmul(psq[:], wqs[:, dt, ts(ht, P)],
                                         xTs[:, dt, ts(mc, 512)],
                                         start=(dt == 0), stop=(dt == D1T - 1))
                    nc.scalar.activation(qT[:, ht, ts(mc, 512)], psq[:],
                                         AF.Identity, bias=bqt[:, ht:ht + 1],
                                         scale=1.0)

        # Stage-C pools (left stack, reusing stage A's range) open now so
        # wvs/vTc prefetch runs during stage B.
        esC = top.enter_context(ExitStack())
        vpool = top.enter_context(tc.tile_pool(name="vres", bufs=1,
                                               side="left"))
        vsb = vpool.tile([P, JT, H], BF16)
        sc1 = esC.enter_context(tc.tile_pool(name="sc1", bufs=1, side="left"))
        sc3 = esC.enter_context(tc.tile_pool(name="sc3", bufs=2, side="left"))
        wvs = sc1.tile([P, D2T, H], BF16)

        # ---- Stage B: kT[h, j] = Wk^T @ Y^T + bk ----
        with tc.tile_pool(name="sbh", bufs=1, side="left") as sbh:
            # Second half of Wk allocates (and loads) as soon as A frees.
            wk_h1 = sbh.tile([P, D2T // 2, H], F32R)
            nc.sync.dma_start(wk_h1[:], wk_t[:, D2T // 2:D2T, :])
            for jc in range(JC):
                yTc = sb3.tile([P, D2T, 512], F32R, tag="yTc")
                nc.sync.dma_start(yTc[:], kyt_t[:, :, ts(jc, 512)])
                if jc == 1:
                    nc.sync.dma_start(wvs[:], wv_t[:])
                for ht in range(HT):
                    psk = pps.tile([P, 512], F32, tag="acc")
                    for dt in range(D2T):
                        wsl = (wk_h0 if dt < D2T // 2 else wk_h1)
                        nc.tensor.matmul(psk[:],
                                         wsl[:, dt % (D2T // 2), ts(ht, P)],
                                         yTc[:, dt, :],
                                         start=(dt == 0), stop=(dt == D2T - 1))
                    nc.scalar.activation(kT[:, ht, ts(jc, 512)], psk[:],
                                         AF.Identity, bias=bkt[:, ht:ht + 1],
                                         scale=1.0)

        # Stage-B pools released (right stack); stage-D pools take their
        # place on the right so softmax buffers prefetch during stage C.
        esB.close()
        esD = top.enter_context(ExitStack())
        sd2 = esD.enter_context(tc.tile_pool(name="sd2", bufs=2, side="right"))
        sd3 = esD.enter_context(tc.tile_pool(name="sd3", bufs=2, side="right"))
        sdc = esD.enter_context(tc.tile_pool(name="sdc", bufs=1, side="right"))
        stat = esD.enter_context(tc.tile_pool(name="stat", bufs=3,
                                              side="right"))
        bv_full = sdc.tile([P, H], F32)
        nc.sync.dma_start(bv_full[:], bvd[None, :].to_broadcast([P, H]))

        # ---- Stage C: v[j, h] = Vin^T-blocks @ Wv (bv folded in at the end) ----
        if True:
            for jc in range(JC):
                vTc = sc3.tile([P, D2T, 512], BF16, tag="vTc")
                nc.sync.dma_start(vTc[:], vvt_t[:, :, ts(jc, 512)])
                for jt4 in range(4):
                    jt = jc * 4 + jt4
                    for hc in range(H // 512):
                        psv = pps.tile([P, 512], F32, tag="acc")
                        for dt in range(D2T):
                            nc.tensor.matmul(psv[:], vTc[:, dt, ts(jt4, P)],
                                             wvs[:, dt, ts(hc, 512)],
                                             start=(dt == 0),
                                             stop=(dt == D2T - 1))
                        nc.vector.tensor_copy(vsb[:, jt, ts(hc, 512)], psv[:])

        # ---- Stage D: per m-tile scores -> softmax -> (probs^T) @ v ----
        # Software-pipelined: AV of m-tile i is emitted after the softmax/
        # transpose of m-tile i+1 has been set in motion.
        esC.close()
        if True:
            def scores_softmax(mt):
                ssb = sd2.tile([P, JC, 512], F32, tag="ssb")
                mx4 = stat.tile([P, JC], F32, tag="mx4")
                for jc in range(JC):
                    pss = pps.tile([P, 512], F32, tag="acc")
                    for ht in range(HT):
                        nc.tensor.matmul(pss[:], qT[:, ht, ts(mt, P)],
                                         kT[:, ht, ts(jc, 512)],
                                         start=(ht == 0), stop=(ht == HT - 1))
                    nc.vector.tensor_copy(ssb[:, jc, :], pss[:])
                    nc.vector.reduce_max(mx4[:, jc:jc + 1], pss[:], axis=AX)
                negmax = stat.tile([P, 1], F32, tag="negmax")
                nc.vector.reduce_max(negmax[:], mx4[:], axis=AX, negate=True)
                wsb = sd2.tile([P, JC, 512], BF16, tag="wsb")
                sm4 = stat.tile([P, JC], F32, tag="sm4")
                for jc in range(JC):
                    nc.scalar.activation(wsb[:, jc, :], ssb[:, jc, :], AF.Exp,
                                         bias=negmax[:, 0:1], scale=1.0,
                                         accum_out=sm4[:, jc:jc + 1])
                ssum = stat.tile([P, 1], F32, tag="ssum")
                nc.vector.reduce_sum(ssum[:], sm4[:], axis=AX)
                rinv = stat.tile([P, 1], F32, tag="rinv")
                nc.vector.reciprocal(rinv[:], ssum[:])
                wT = sd3.tile([P, JT, P], BF16, tag="wT")
                nc.scalar.dma_start_transpose(
                    wT[:], wsb[:].rearrange("p a b -> p (a b)"))
                return wT, rinv

            def av(mt, wT, rinv):
                osb = sd2.tile([P, H], F32, tag="osb")
                for hc in range(H // 512):
                    psa = pps.tile([P, 512], F32, tag="acc")
                    for jt in range(JT):
                        nc.tensor.matmul(psa[:], wT[:, jt, :],
                                         vsb[:, jt, ts(hc, 512)],
                                         start=(jt == 0), stop=(jt == JT - 1))
                    nc.scalar.activation(osb[:, ts(hc, 512)], psa[:], AF.Copy,
                                         scale=rinv[:, 0:1])
                nc.vector.tensor_tensor(osb[:], osb[:], bv_full[:], ALU.add)
                nc.sync.dma_start(out[ts(mt, P), :], osb[:])

            prev = None
            for mt in range(MT):
                cur = scores_softmax(mt)
                if prev is not None:
                    av(prev[0], prev[1], prev[2])
                prev = (mt,) + cur
            av(prev[0], prev[1], prev[2])

    nc.compile()
    return nc


def _get_nc():
    if "nc" not in _CACHE:
        _CACHE["nc"] = _build_bass()
    return _CACHE["nc"]


def kernel(query, key, value, Wq, bq, Wk, bk, Wv, bv):
    global LAST_RESULTS
    nc = _get_nc()

    def f(a):
        return np.ascontiguousarray(np.asarray(a, dtype=np.float32))

    query, key, value = f(query), f(key), f(value)
    Wq, bq, Wk, bk, Wv, bv = f(Wq), f(bq), f(Wk), f(bk), f(Wv), f(bv)

    in_maps = []
    half = LQ // 2
    import ml_dtypes
    keyT = [np.ascontiguousarray(key[b].T) for b in range(B)]
    valT = [np.ascontiguousarray(value[b].T.astype(ml_dtypes.bfloat16))
            for b in range(B)]
    Wv = Wv.astype(ml_dtypes.bfloat16)
    for c in range(N_CORES):
        b, h = divmod(c, 2)
        in_maps.append({
            "xqt": np.ascontiguousarray(query[b, h * half:(h + 1) * half, :].T),
            "kyt": keyT[b],
            "vvt": valT[b],
            "wq": Wq, "wk": Wk, "wv": Wv,
            "bq": bq, "bk": bk, "bv": bv,
        })

    res = run_bass_kernel_spmd(nc, in_maps, core_ids=list(range(N_CORES)))
    LAST_RESULTS = res

    out = np.empty((B, LQ, H), dtype=np.float32)
    for c in range(N_CORES):
        b, h = divmod(c, 2)
        out[b, h * half:(h + 1) * half, :] = res.results[c]["out"]
    return out



# revision 4
# speedup vs baseline: 1.6064x; 1.6064x over previous
"""CrossAttention Trainium2 kernel (Bass/Tile), 8-core SPMD.

Problem: q = query@Wq+bq; k = key@Wk+bk; v = value@Wv+bv;
         out = softmax(q k^T) v           (no 1/sqrt(d) scaling)
Shapes:  query [4, 2048, 1024], key/value [4, 2048, 768],
         W* [(1024|768), 1024], b* [1024], out [4, 2048, 1024] f32.

Sharding: data-parallel over (batch, query-half) -> 8 shards of 1024 query
rows per core. No collectives.

Algebraic restructuring (the big win): softmax is invariant to per-row
constants, so
  scores  = (x Wq + bq)(y Wk + bk)^T
         == x (Wq Wk^T) y^T + t[j],   t = key @ (Wk @ bq)   (host-computed)
which deletes both the Q and K projections (the H=1024 contraction
collapses into the host-precomputed Wqk, and the scores contraction
shrinks H=1024 -> D2=768). The V side is re-associated as
  out = (probs @ value) @ Wv + bv
so the Wv GEMM runs on [M=1024, 768] instead of [LK=2048, 768].
Per-core PE work drops from ~218us (baseline) to ~123us.

Precision: scores path f32r end-to-end (logits are sigma~32; bf16 there
costs ~0.1 logit of noise and softmax ties amplify it). V path bf16
(fp8 measured at rel-err 4.6e-2 - the values' own quantization noise -
so fp8 is banned everywhere).

Stage D is software-pipelined: AVy of m-tile i is emitted after scores of
m-tile i+2, AVw after scores of m-tile i+3, giving the softmax->transpose
->AVy->transpose chains multiple score-windows of slack.
"""

import os
import sys
from contextlib import ExitStack

for _p in ("/opt/trn_rl_repo", "/root/.axon_site/_ro/trn_rl_repo"):
    if os.path.isdir(_p) and _p not in sys.path:
        sys.path.append(_p)

import numpy as np

import concourse.bass as bass
import concourse.mybir as mybir
import concourse.tile as tile
from concourse import bacc
from concourse.bass import ts
from concourse.bass_utils import run_bass_kernel_spmd

P = 128
B, LQ, LK = 4, 2048, 2048
D1, D2, H = 1024, 768, 1024
N_CORES = 8
M = (B * LQ) // N_CORES  # 1024 query rows per core

D1T, D2T, HT = D1 // P, D2 // P, H // P
MT, JT, JC, MC = M // P, LK // P, LK // 512, M // 512
ET = D2 // P  # 6 tiles of the D2 contraction/intermediate axis

F32 = mybir.dt.float32
F32R = mybir.dt.float32r
BF16 = mybir.dt.bfloat16
AX = mybir.AxisListType.X
AF = mybir.ActivationFunctionType
ALU = mybir.AluOpType

_CACHE = {}
LAST_RESULTS = None  # BassKernelResults of the most recent run (for test harness)


def _build_bass():
    nc = bacc.Bacc("TRN2", target_bir_lowering=False, debug=False,
                   num_devices=N_CORES)

    # Host-pre-tiled operands: every DMA chunk below is contiguous.
    xq = nc.dram_tensor("xq", [P, MC, D1T, 512], F32R, kind="ExternalInput")
    wqkd = nc.dram_tensor("wqk", [P, ET, D1T, P], F32R, kind="ExternalInput")
    kyd = nc.dram_tensor("ky", [P, JC, ET, 512], F32R, kind="ExternalInput")
    yvd = nc.dram_tensor("yv", [P, JT, D2], BF16, kind="ExternalInput")
    wvd = nc.dram_tensor("wv", [P, ET, H], BF16, kind="ExternalInput")
    tqd = nc.dram_tensor("tq", [LK], F32, kind="ExternalInput")
    bvd = nc.dram_tensor("bv", [H], F32, kind="ExternalInput")
    out = nc.dram_tensor("out", [M, H], F32, kind="ExternalOutput")

    with tile.TileContext(nc) as tc, ExitStack() as top:
        # Shared PSUM pools: "acc" for scores/Z/AVw, py1/py2 for AVy.
        pps = top.enter_context(tc.tile_pool(name="pps", bufs=4, space="PSUM"))
        ppy = top.enter_context(tc.tile_pool(name="ppy", bufs=2, space="PSUM"))

        # Residents: zT [768, M] f32r, yT [768, LK] f32r, yv [LK, 768] bf16,
        # wv [768, H] bf16, t/bv broadcasts.
        respool = top.enter_context(tc.tile_pool(name="res", bufs=1))
        zT = respool.tile([P, ET, M], F32R)
        yTs = respool.tile([P, ET, LK], F32R)
        yv = respool.tile([P, JT, D2], BF16)
        wvs = respool.tile([P, ET, H], BF16)
        tsb = respool.tile([P, JC, 512], F32)
        bv_full = respool.tile([P, H], F32)

        # ---- Stage Z: zT[e, m] = Wqk^T @ X^T  (f32r) ----
        # Need-ordered queue: xq halves + per-et Wqk chunks feed the PE from
        # ~17us; yT/yv/wv stream in behind while Z and early scores run.
        with tc.tile_pool(name="sa1", bufs=1, side="left") as sa1, \
                tc.tile_pool(name="saw", bufs=3, side="left") as saw:
            xTs = sa1.tile([P, MC, D1T, 512], F32R)
            nc.sync.dma_start(xTs[:, 0], xq[:, 0])
            for et in range(ET):
                wqkc = saw.tile([P, D1T, P], F32R, tag="wqkc")
                nc.sync.dma_start(wqkc[:], wqkd[:, et])
                if et == 0:
                    nc.sync.dma_start(xTs[:, 1], xq[:, 1])
                for mc in range(MC):
                    psz = pps.tile([P, 512], F32, tag="acc")
                    for dt in range(D1T):
                        nc.tensor.matmul(psz[:], wqkc[:, dt, :],
                                         xTs[:, mc, dt, :],
                                         start=(dt == 0), stop=(dt == D1T - 1))
                    nc.scalar.activation(zT[:, et, ts(mc, 512)], psz[:],
                                         AF.Copy, scale=1.0)
            # Stage-D operand stream, in need order.
            nc.sync.dma_start(tsb[:].rearrange("p a b -> p (a b)"),
                              tqd[None, :].to_broadcast([P, LK]))
            for jc in range(JC):
                nc.sync.dma_start(yTs[:, :, ts(jc, 512)], kyd[:, jc])
            for c4 in range(4):
                nc.sync.dma_start(yv[:, 4 * c4:4 * c4 + 4, :],
                                  yvd[:, 4 * c4:4 * c4 + 4, :])
            nc.sync.dma_start(wvs[:], wvd[:])
            nc.sync.dma_start(bv_full[:], bvd[None, :].to_broadcast([P, H]))

        # ---- Stage D pools ----
        esD = top.enter_context(ExitStack())
        sd2 = esD.enter_context(tc.tile_pool(name="sd2", bufs=3, side="right"))
        sd3 = esD.enter_context(tc.tile_pool(name="sd3", bufs=3, side="right"))
        sdz = esD.enter_context(tc.tile_pool(name="sdz", bufs=2, side="right"))
        stat = esD.enter_context(tc.tile_pool(name="stat", bufs=4,
                                              side="right"))

        # ---- Stage D: scores -> softmax -> (probs @ value) @ Wv ----
        def scores_softmax(mt):
            ssb = sd2.tile([P, JC, 512], F32, tag="ssb")
            mx4 = stat.tile([P, JC], F32, tag="mx4")
            for jc in range(JC):
                pss = pps.tile([P, 512], F32, tag="acc")
                for et in range(ET):
                    nc.tensor.matmul(pss[:], zT[:, et, ts(mt, P)],
                                     yTs[:, et, ts(jc, 512)],
                                     start=(et == 0), stop=(et == ET - 1))
                # copy + fold the per-key bias t[j] in one DVE pass
                nc.vector.tensor_tensor(ssb[:, jc, :], pss[:], tsb[:, jc, :],
                                        ALU.add)
                nc.vector.reduce_max(mx4[:, jc:jc + 1], ssb[:, jc, :], axis=AX)
            negmax = stat.tile([P, 1], F32, tag="negmax")
            nc.vector.reduce_max(negmax[:], mx4[:], axis=AX, negate=True)
            wsb = sd2.tile([P, JC, 512], BF16, tag="wsb")
            sm4 = stat.tile([P, JC], F32, tag="sm4")
            for jc in range(JC):
                nc.scalar.activation(wsb[:, jc, :], ssb[:, jc, :], AF.Exp,
                                     bias=negmax[:, 0:1], scale=1.0,
                                     accum_out=sm4[:, jc:jc + 1])
            ssum = stat.tile([P, 1], F32, tag="ssum")
            nc.vector.reduce_sum(ssum[:], sm4[:], axis=AX)
            rinv = stat.tile([P, 1], F32, tag="rinv")
            nc.vector.reciprocal(rinv[:], ssum[:])
            wT = sd3.tile([P, JT, P], BF16, tag="wT")
            nc.scalar.dma_start_transpose(
                wT[:], wsb[:].rearrange("p a b -> p (a b)"))
            return wT, rinv

        def avy(mt, wT):
            # zy[m, e] = sum_j probs^T[j, m] * value[j, e]   (bf16)
            py1 = ppy.tile([P, 512], F32, tag="py1")
            py2 = ppy.tile([P, 256], F32, tag="py2")
            for jt in range(JT):
                nc.tensor.matmul(py1[:], wT[:, jt, :], yv[:, jt, 0:512],
                                 start=(jt == 0), stop=(jt == JT - 1))
                nc.tensor.matmul(py2[:], wT[:, jt, :], yv[:, jt, 512:D2],
                                 start=(jt == 0), stop=(jt == JT - 1))
            zy = sdz.tile([P, D2], BF16, tag="zy")
            nc.scalar.activation(zy[:, 0:512], py1[:], AF.Copy, scale=1.0)
            nc.scalar.activation(zy[:, 512:D2], py2[:], AF.Copy, scale=1.0)
            zyT = sdz.tile([P, ET, P], BF16, tag="zyT")
            nc.scalar.dma_start_transpose(zyT[:], zy[:])
            return zyT

        def avw(mt, zyT, rinv):
            osb = sd2.tile([P, H], F32, tag="osb")
            for hc in range(H // 512):
                psa = pps.tile([P, 512], F32, tag="acc")
                for et in range(ET):
                    nc.tensor.matmul(psa[:], zyT[:, et, :],
                                     wvs[:, et, ts(hc, 512)],
                                     start=(et == 0), stop=(et == ET - 1))
                nc.scalar.activation(osb[:, ts(hc, 512)], psa[:], AF.Copy,
                                     scale=rinv[:, 0:1])
            nc.vector.tensor_tensor(osb[:], osb[:], bv_full[:], ALU.add)
            nc.sync.dma_start(out[ts(mt, P), :], osb[:])

        penda = []  # (mt, wT, rinv) awaiting AVy
        pendw = []  # (mt, zyT, rinv) awaiting AVw
        for mt in range(MT):
            penda.append((mt,) + scores_softmax(mt))
            if len(penda) > 2:
                amt, wT, rinv = penda.pop(0)
                pendw.append((amt, avy(amt, wT), rinv))
            if len(pendw) > 1:
                wmt, zyT, rinv = pendw.pop(0)
                avw(wmt, zyT, rinv)
        while penda or pendw:
            if penda:
                amt, wT, rinv = penda.pop(0)
                pendw.append((amt, avy(amt, wT), rinv))
            wmt, zyT, rinv = pendw.pop(0)
            avw(wmt, zyT, rinv)

    nc.compile()
    return nc


def _get_nc():
    if "nc" not in _CACHE:
        _CACHE["nc"] = _build_bass()
    return _CACHE["nc"]


def kernel(query, key, value, Wq, bq, Wk, bk, Wv, bv):
    global LAST_RESULTS
    nc = _get_nc()
    import ml_dtypes

    def f(a):
        return np.ascontiguousarray(np.asarray(a, dtype=np.float32))

    query, key, value = f(query), f(key), f(value)
    Wq, bq, Wk, bk, Wv, bv = f(Wq), f(bq), f(Wk), f(bk), f(Wv), f(bv)

    # Host-side algebra (f64): Wqk = Wq Wk^T ; t = key @ (Wk bq).
    Wqk = (Wq.astype(np.float64) @ Wk.astype(np.float64).T).astype(np.float32)
    wkbq = Wk.astype(np.float64) @ bq.astype(np.float64)  # [D2]
    tq = [(key[b].astype(np.float64) @ wkbq).astype(np.float32)
          for b in range(B)]

    def tile_lhs(w, kt, nb):  # [K, N] -> [P, nb, kt, N/nb] chunk-contiguous
        n = w.shape[1]
        return np.ascontiguousarray(
            w.reshape(kt, P, nb, n // nb).transpose(1, 2, 0, 3))

    wqk_t = tile_lhs(Wqk, D1T, ET)                     # [P, ET, D1T, 128]
    wv_t = np.ascontiguousarray(
        Wv.astype(ml_dtypes.bfloat16).reshape(ET, P, H).transpose(1, 0, 2))

    half = LQ // 2
    ky_t = [tile_lhs(key[b].T.copy(), ET, JC) for b in range(B)]
    yv_t = [np.ascontiguousarray(
        value[b].astype(ml_dtypes.bfloat16).reshape(JT, P, D2)
        .transpose(1, 0, 2)) for b in range(B)]

    in_maps = []
    for c in range(N_CORES):
        b, hh = divmod(c, 2)
        xqT = query[b, hh * half:(hh + 1) * half, :].T  # [D1, M]
        xq_t = np.ascontiguousarray(
            xqT.reshape(D1T, P, MC, 512).transpose(1, 2, 0, 3))
        in_maps.append({
            "xq": xq_t, "wqk": wqk_t, "ky": ky_t[b], "yv": yv_t[b],
            "wv": wv_t, "tq": tq[b], "bv": bv,
        })

    res = run_bass_kernel_spmd(nc, in_maps, core_ids=list(range(N_CORES)))
    LAST_RESULTS = res

    out = np.empty((B, LQ, H), dtype=np.float32)
    for c in range(N_CORES):
        b, hh = divmod(c, 2)
        out[b, hh * half:(hh + 1) * half, :] = res.results[c]["out"]
    return out


# revision 7
# speedup vs baseline: 1.7039x; 1.0607x over previous
"""CrossAttention Trainium2 kernel (Bass/Tile), 8-core SPMD.

Problem: q = query@Wq+bq; k = key@Wk+bk; v = value@Wv+bv;
         out = softmax(q k^T) v           (no 1/sqrt(d) scaling)
Shapes:  query [4, 2048, 1024], key/value [4, 2048, 768],
         W* [(1024|768), 1024], b* [1024], out [4, 2048, 1024] f32.

Sharding: data-parallel over (batch, query-half) -> 8 shards of 1024 query
rows per core. No collectives.

Algebraic restructuring (the big win): softmax is invariant to per-row
constants, so
  scores  = (x Wq + bq)(y Wk + bk)^T
         == x (Wq Wk^T) y^T + t[j],   t = key @ (Wk @ bq)   (host-computed)
which deletes both the Q and K projections (the H=1024 contraction
collapses into the host-precomputed Wqk, and the scores contraction
shrinks H=1024 -> D2=768). The V side is re-associated as
  out = (probs @ value) @ Wv + bv
so the Wv GEMM runs on [M=1024, 768] instead of [LK=2048, 768].
Per-core PE work drops from ~218us (baseline) to ~123us.

Precision: scores path f32r end-to-end (logits are sigma~32; bf16 there
costs ~0.1 logit of noise and softmax ties amplify it). V path bf16
(fp8 measured at rel-err 4.6e-2 - the values' own quantization noise -
so fp8 is banned everywhere).

Stage D is software-pipelined: AVy of m-tile i is emitted after scores of
m-tile i+2, AVw after scores of m-tile i+3, giving the softmax->transpose
->AVy->transpose chains multiple score-windows of slack.
"""

import os
import sys
from contextlib import ExitStack

for _p in ("/opt/trn_rl_repo", "/root/.axon_site/_ro/trn_rl_repo"):
    if os.path.isdir(_p) and _p not in sys.path:
        sys.path.append(_p)

import numpy as np

import concourse.bass as bass
import concourse.mybir as mybir
import concourse.tile as tile
from concourse import bacc
from concourse.bass import ts
from concourse.bass_utils import run_bass_kernel_spmd

P = 128
B, LQ, LK = 4, 2048, 2048
D1, D2, H = 1024, 768, 1024
N_CORES = 8
M = (B * LQ) // N_CORES  # 1024 query rows per core

D1T, D2T, HT = D1 // P, D2 // P, H // P
MT, JT, JC, MC = M // P, LK // P, LK // 512, M // 512
ET = D2 // P  # 6 tiles of the D2 contraction/intermediate axis

F32 = mybir.dt.float32
F32R = mybir.dt.float32r
BF16 = mybir.dt.bfloat16
AX = mybir.AxisListType.X
AF = mybir.ActivationFunctionType
ALU = mybir.AluOpType

_CACHE = {}
LAST_RESULTS = None  # BassKernelResults of the most recent run (for test harness)


def _build_bass():
    nc = bacc.Bacc("TRN2", target_bir_lowering=False, debug=False,
                   num_devices=N_CORES)

    # Host-pre-tiled operands: every DMA chunk below is contiguous.
    xq = nc.dram_tensor("xq", [P, MC, D1T, 512], F32R, kind="ExternalInput")
    wqkd = nc.dram_tensor("wqk", [P, ET, D1T, P], F32R, kind="ExternalInput")
    kyd = nc.dram_tensor("ky", [P, JC, ET, 512], F32R, kind="ExternalInput")
    yvd = nc.dram_tensor("yv", [P, JT, D2], BF16, kind="ExternalInput")
    wvd = nc.dram_tensor("wv", [P, ET, H], BF16, kind="ExternalInput")
    tqd = nc.dram_tensor("tq", [LK], F32, kind="ExternalInput")
    bvd = nc.dram_tensor("bv", [H], F32, kind="ExternalInput")
    out = nc.dram_tensor("out", [M, H], F32, kind="ExternalOutput")

    with tile.TileContext(nc) as tc, ExitStack() as top:
        # Shared PSUM pools: "acc" for scores/Z/AVw, py1/py2 for AVy.
        pps = top.enter_context(tc.tile_pool(name="pps", bufs=4, space="PSUM"))
        ppy = top.enter_context(tc.tile_pool(name="ppy", bufs=2, space="PSUM"))

        # Residents: zT [768, M] f32r, yT [768, LK] f32r, yv [LK, 768] bf16,
        # wv [768, H] bf16, t/bv broadcasts.
        respool = top.enter_context(tc.tile_pool(name="res", bufs=1))
        zT = respool.tile([P, ET, M], F32R)
        yTs = respool.tile([P, ET, LK], F32R)
        yv = respool.tile([P, JT, D2], BF16)
        wvs = respool.tile([P, ET, H], BF16)
        tsb = respool.tile([P, JC, 512], F32)
        bv_full = respool.tile([P, H], F32)

        # ---- Stage Z: zT[e, m] = Wqk^T @ X^T  (f32r) ----
        # Need-ordered queue: xq halves + per-et Wqk chunks feed the PE from
        # ~17us; yT/yv/wv stream in behind while Z and early scores run.
        with tc.tile_pool(name="sa1", bufs=1, side="left") as sa1, \
                tc.tile_pool(name="saw", bufs=6, side="left") as saw:
            xTs = sa1.tile([P, MC, D1T, 512], F32R)
            nc.sync.dma_start(xTs[:, 0], xq[:, 0])
            wqkcs = []
            for et in range(ET):
                wqkc = saw.tile([P, D1T, P], F32R, tag="wqkc")
                nc.sync.dma_start(wqkc[:], wqkd[:, et])
                wqkcs.append(wqkc)
            nc.sync.dma_start(xTs[:, 1], xq[:, 1])
            # Stage-D operand stream queued behind Z's operands, in need order.
            for jc in range(2):
                nc.sync.dma_start(yTs[:, :, ts(jc, 512)], kyd[:, jc])
            nc.sync.dma_start(tsb[:].rearrange("p a b -> p (a b)"),
                              tqd[None, :].to_broadcast([P, LK]))
            for jc in range(2, JC):
                nc.sync.dma_start(yTs[:, :, ts(jc, 512)], kyd[:, jc])
            for c4 in range(4):
                nc.sync.dma_start(yv[:, 4 * c4:4 * c4 + 4, :],
                                  yvd[:, 4 * c4:4 * c4 + 4, :])
            nc.sync.dma_start(wvs[:], wvd[:])
            nc.sync.dma_start(bv_full[:], bvd[None, :].to_broadcast([P, H]))
            for mc in range(MC):
                for et in range(ET):
                    psz = pps.tile([P, 512], F32, tag="acc")
                    for dt in range(D1T):
                        nc.tensor.matmul(psz[:], wqkcs[et][:, dt, :],
                                         xTs[:, mc, dt, :],
                                         start=(dt == 0), stop=(dt == D1T - 1))
                    nc.scalar.activation(zT[:, et, ts(mc, 512)], psz[:],
                                         AF.Copy, scale=1.0)

        # ---- Stage D pools ----
        esD = top.enter_context(ExitStack())
        sd2 = esD.enter_context(tc.tile_pool(name="sd2", bufs=3, side="right"))
        sd3 = esD.enter_context(tc.tile_pool(name="sd3", bufs=3, side="right"))
        sdz = esD.enter_context(tc.tile_pool(name="sdz", bufs=3, side="right"))
        stat = esD.enter_context(tc.tile_pool(name="stat", bufs=4,
                                              side="right"))

        # ---- Stage D: scores -> softmax -> (probs @ value) @ Wv ----
        def scores_softmax(mt):
            ssb = sd2.tile([P, JC, 512], F32, tag="ssb")
            mx4 = stat.tile([P, JC], F32, tag="mx4")
            for jc in range(JC):
                pss = pps.tile([P, 512], F32, tag="acc")
                for et in range(ET):
                    nc.tensor.matmul(pss[:], zT[:, et, ts(mt, P)],
                                     yTs[:, et, ts(jc, 512)],
                                     start=(et == 0), stop=(et == ET - 1))
                # copy + fold the per-key bias t[j] in one DVE pass
                nc.vector.tensor_tensor(ssb[:, jc, :], pss[:], tsb[:, jc, :],
                                        ALU.add)
                nc.vector.reduce_max(mx4[:, jc:jc + 1], ssb[:, jc, :], axis=AX)
            negmax = stat.tile([P, 1], F32, tag="negmax")
            nc.vector.reduce_max(negmax[:], mx4[:], axis=AX, negate=True)
            wsb = sd2.tile([P, JC, 512], BF16, tag="wsb")
            sm4 = stat.tile([P, JC], F32, tag="sm4")
            for jc in range(JC):
                nc.scalar.activation(wsb[:, jc, :], ssb[:, jc, :], AF.Exp,
                                     bias=negmax[:, 0:1], scale=1.0,
                                     accum_out=sm4[:, jc:jc + 1])
            ssum = stat.tile([P, 1], F32, tag="ssum")
            nc.vector.reduce_sum(ssum[:], sm4[:], axis=AX)
            rinv = stat.tile([P, 1], F32, tag="rinv")
            nc.vector.reciprocal(rinv[:], ssum[:])
            wT = sd3.tile([P, JT, P], BF16, tag="wT")
            nc.sync.dma_start_transpose(
                wT[:], wsb[:].rearrange("p a b -> p (a b)"))
            return wT, rinv

        def avy(mt, wT):
            # zy[m, e] = sum_j probs^T[j, m] * value[j, e]   (bf16)
            # 384/384 split keeps every matmul >= LDWEIGHTS time.
            py1 = ppy.tile([P, 384], F32, tag="py1")
            py2 = ppy.tile([P, 384], F32, tag="py2")
            for jt in range(JT):
                nc.tensor.matmul(py1[:], wT[:, jt, :], yv[:, jt, 0:384],
                                 start=(jt == 0), stop=(jt == JT - 1))
                nc.tensor.matmul(py2[:], wT[:, jt, :], yv[:, jt, 384:D2],
                                 start=(jt == 0), stop=(jt == JT - 1))
            zy = sdz.tile([P, D2], BF16, tag="zy")
            nc.vector.tensor_copy(zy[:, 0:384], py1[:])
            nc.vector.tensor_copy(zy[:, 384:D2], py2[:])
            zyT = sdz.tile([P, ET, P], BF16, tag="zyT")
            nc.sync.dma_start_transpose(zyT[:], zy[:])
            return zyT

        def avw(mt, zyT, rinv):
            osb = sd2.tile([P, H], F32, tag="osb")
            for hc in range(H // 512):
                psa = pps.tile([P, 512], F32, tag="acc")
                for et in range(ET):
                    nc.tensor.matmul(psa[:], zyT[:, et, :],
                                     wvs[:, et, ts(hc, 512)],
                                     start=(et == 0), stop=(et == ET - 1))
                nc.scalar.activation(osb[:, ts(hc, 512)], psa[:], AF.Copy,
                                     scale=rinv[:, 0:1])
                nc.vector.tensor_tensor(osb[:, ts(hc, 512)],
                                        osb[:, ts(hc, 512)],
                                        bv_full[:, ts(hc, 512)], ALU.add)
                nc.sync.dma_start(out[ts(mt, P), ts(hc, 512)],
                                  osb[:, ts(hc, 512)])

        penda = []  # (mt, wT, rinv) awaiting AVy
        pendw = []  # (mt, zyT, rinv) awaiting AVw
        for mt in range(MT):
            penda.append((mt,) + scores_softmax(mt))
            if len(penda) > 2:
                amt, wT, rinv = penda.pop(0)
                pendw.append((amt, avy(amt, wT), rinv))
            if len(pendw) > 1:
                wmt, zyT, rinv = pendw.pop(0)
                avw(wmt, zyT, rinv)
        while penda or pendw:
            if penda:
                amt, wT, rinv = penda.pop(0)
                pendw.append((amt, avy(amt, wT), rinv))
            wmt, zyT, rinv = pendw.pop(0)
            avw(wmt, zyT, rinv)

    nc.compile()
    return nc


def _get_nc():
    if "nc" not in _CACHE:
        _CACHE["nc"] = _build_bass()
    return _CACHE["nc"]


def kernel(query, key, value, Wq, bq, Wk, bk, Wv, bv):
    global LAST_RESULTS
    nc = _get_nc()
    import ml_dtypes

    def f(a):
        return np.ascontiguousarray(np.asarray(a, dtype=np.float32))

    query, key, value = f(query), f(key), f(value)
    Wq, bq, Wk, bk, Wv, bv = f(Wq), f(bq), f(Wk), f(bk), f(Wv), f(bv)

    # Host-side algebra (f64): Wqk = Wq Wk^T ; t = key @ (Wk bq).
    Wqk = (Wq.astype(np.float64) @ Wk.astype(np.float64).T).astype(np.float32)
    wkbq = Wk.astype(np.float64) @ bq.astype(np.float64)  # [D2]
    tq = [(key[b].astype(np.float64) @ wkbq).astype(np.float32)
          for b in range(B)]

    def tile_lhs(w, kt, nb):  # [K, N] -> [P, nb, kt, N/nb] chunk-contiguous
        n = w.shape[1]
        return np.ascontiguousarray(
            w.reshape(kt, P, nb, n // nb).transpose(1, 2, 0, 3))

    wqk_t = tile_lhs(Wqk, D1T, ET)                     # [P, ET, D1T, 128]
    wv_t = np.ascontiguousarray(
        Wv.astype(ml_dtypes.bfloat16).reshape(ET, P, H).transpose(1, 0, 2))

    half = LQ // 2
    ky_t = [tile_lhs(key[b].T.copy(), ET, JC) for b in range(B)]
    yv_t = [np.ascontiguousarray(
        value[b].astype(ml_dtypes.bfloat16).reshape(JT, P, D2)
        .transpose(1, 0, 2)) for b in range(B)]

    in_maps = []
    for c in range(N_CORES):
        b, hh = divmod(c, 2)
        xqT = query[b, hh * half:(hh + 1) * half, :].T  # [D1, M]
        xq_t = np.ascontiguousarray(
            xqT.reshape(D1T, P, MC, 512).transpose(1, 2, 0, 3))
        in_maps.append({
            "xq": xq_t, "wqk": wqk_t, "ky": ky_t[b], "yv": yv_t[b],
            "wv": wv_t, "tq": tq[b], "bv": bv,
        })

    res = run_bass_kernel_spmd(nc, in_maps, core_ids=list(range(N_CORES)))
    LAST_RESULTS = res

    out = np.empty((B, LQ, H), dtype=np.float32)
    for c in range(N_CORES):
        b, hh = divmod(c, 2)
        out[b, hh * half:(hh + 1) * half, :] = res.results[c]["out"]
    return out
